# revision 11
# baseline (speedup 1.0000x reference)
"""GAT (2-layer dense graph attention) on 8 Trainium2 NeuronCores.

Sharding: nodes (rows) split 8 ways, 750 rows/core, column-on-partition
("transposed") layout so attention probabilities feed TensorE directly.

v3 vs the 845us baseline:
  - BOTH branches of exp(lrelu(s) - T) are separable outer products:
      exp(s - T)   = exp(f1_i - c1) * exp(f2_j - c2)
      exp(.2s - T) = exp(.2f1_i - c1) * exp(.2f2_j - c2)   (c1 + c2 = T)
    so u = max(a1*b1, a2*b2) needs NO per-tile exp. Shifts c1=c2=4 (layer 1),
    1.5 (layer 2) sized from the reference logit ranges (|f|<6.7, |g|<1.3) so
    every fp16 intermediate is in range (factors <= e^1.8, 1/rowsum <= e^10.7).
  - attention aggregation + rowsum matmuls run in fp8 DoubleRow perf mode
    (2 j-tiles per PE pass): p in e5m2, Wh/Wh2 stationaries in e4m3.
    Measured end-to-end error of this mix on the reference inputs: 6.4e-3.
  - per-tile elementwise work round-robins across DVE (tensor_scalar +
    max + mask), ScalarE (prelu+exp form), GpSimd (max or t2+max, fp16 only)
    to balance the three engines; psum->sbuf copies also rotate engines.
  - phase-B elementwise SBUF pools are allocated before phase A so the B
    work overlaps A's PE-bound projection phase (no pool-boundary stall).
  - hc stays in SBUF feeding the Wh2 accumulation per head; the ELU "-1" is
    folded into a rank-1 Wout column-sum correction on the psW2 copy-out.
  - g2 is computed from local Wh2 rows and gathered in a small collective
    BEFORE the big (fp8) Wh2 AllGather; all layer-2 elementwise + rowsums
    overlap the big gather, aggregation runs once it lands.
"""

import sys
import numpy as np

sys.path.insert(0, "/opt/trn_rl_repo")

N = 6000
F_IN = 300
HID = 128
H = 8
NC = 8          # cores
R = 750         # rows per core
JT = 47         # j tiles of 128 (6016 padded)
NPAD = JT * 128  # 6016
KP = 384        # padded F_IN (3 chunks of 128)
NH = H * HID    # 1024
ALPHA = 0.2
C1 = 4.0        # layer-1 exp shift split (c1 + c2 = SHIFT1)
SHIFT1 = 8.0
CD = 1.5        # layer-2 shift split
SHIFT2 = 3.0
PR = 768        # padded pair-tile row length (stride % 16 == 0)

_CACHE = {}


def _form(jt):
    """Engine assignment per j-tile: S = ScalarE prelu+exp, G = GpSimd max,
    F = GpSimd t2+max, else DVE-only."""
    if jt % 8 == 1:
        return "F"
    return "S" if jt % 2 == 0 else "G"


def _mm_acc(nc, psum, lhsT, rhs, start, stop, width=512, perf_mode=None):
    n = rhs.shape[-1]
    for lo in range(0, n, width):
        hi = min(lo + width, n)
        nc.tensor.matmul(psum[:, lo:hi], lhsT, rhs[..., lo:hi],
                         start=start, stop=stop, perf_mode=perf_mode)


def _bcast_row(bass, row_ap, parts=128):
    """AP that reads a [1, n] DRAM row replicated across `parts` partitions."""
    return bass.AP(tensor=row_ap.tensor, offset=row_ap.offset,
                   ap=[[0, parts]] + [list(d) for d in row_ap.ap[1:]])


def _dram_ap(bass, tile_ap, offset_elems, dims):
    """Manual AP over a DRAM tile at an element offset with [stride,count] dims."""
    return bass.AP(tensor=tile_ap.tensor, offset=tile_ap.offset + offset_elems,
                   ap=[list(d) for d in dims])


def _build(reps=1):
    import concourse.bass as bass
    import concourse.tile as tile
    import concourse.tile_utils as tile_utils
    from concourse import bacc, mybir
    from concourse.masks import make_identity

    tile_utils.max_sbuf_usage = 206 * 1024

    f32, f16 = mybir.dt.float32, mybir.dt.float16
    f8e4, f8e5 = mybir.dt.float8e4, mybir.dt.float8e5
    AF = mybir.ActivationFunctionType
    ALU = mybir.AluOpType

    nc = bacc.Bacc("TRN2", target_bir_lowering=False, debug=False,
                   enable_asserts=False, num_devices=NC)

    xT16 = nc.dram_tensor("xT16", [KP, NPAD], f16, kind="ExternalInput")
    xTl16 = nc.dram_tensor("xTl16", [KP, R], f16, kind="ExternalInput")
    W16 = nc.dram_tensor("W16", [KP, NH], f16, kind="ExternalInput")
    WT32 = nc.dram_tensor("WT32", [HID, H, KP], f32, kind="ExternalInput")
    A12 = nc.dram_tensor("a12", [HID, H, 2], f32, kind="ExternalInput")
    AOB16 = nc.dram_tensor("aob16", [HID, 2], f16, kind="ExternalInput")
    WO16 = nc.dram_tensor("Wout16", [NH, HID], f16, kind="ExternalInput")
    ADJT = nc.dram_tensor("adjT", [NPAD, R], f16, kind="ExternalInput")
    OUT = nc.dram_tensor("out", [R, HID], f32, kind="ExternalOutput")

    with tile.TileContext(nc) as tc:
        for rep in range(reps):
            _body(nc, tc, bass, tile, mybir, f32, f16, f8e4, f8e5, AF, ALU,
                  make_identity, xT16, xTl16, W16, WT32, A12, AOB16, WO16,
                  ADJT, OUT, pfx=f"r{rep}_" if reps > 1 else "")
    nc.compile()
    return nc


def _body(nc, tc, bass, tile, mybir, f32, f16, f8e4, f8e5, AF, ALU,
          make_identity, xT16, xTl16, W16, WT32, A12, AOB16, WO16, ADJT, OUT,
          pfx=""):
    DR = mybir.MatmulPerfMode.DoubleRow
    whtp_cm = tc.tile_pool(name=pfx + "whtp", bufs=1, side="right")
    whtp = whtp_cm.__enter__()
    with tc.tile_pool(name=pfx + "persist", bufs=1) as persist, \
         tc.tile_pool(name=pfx + "dram", bufs=1, space="DRAM") as dram:

        ident32 = persist.tile([128, 128], f32)
        make_identity(nc, ident32)
        ident16 = persist.tile([128, 128], f16)
        nc.vector.tensor_copy(out=ident16, in_=ident32)
        ones8 = persist.tile([128, 2, 16], f8e4)
        nc.vector.memset(ones8, 1.0)
        zero_b = persist.tile([128, 1], f32)
        nc.vector.memset(zero_b, 0.0)
        nSH1_b = persist.tile([128, 1], f32)
        nc.vector.memset(nSH1_b, -SHIFT1)
        nSH2_b = persist.tile([128, 1], f32)
        nc.vector.memset(nSH2_b, -SHIFT2)
        nC1_b = persist.tile([128, 1], f32)
        nc.vector.memset(nC1_b, -C1)
        nCD_b = persist.tile([128, 1], f32)
        nc.vector.memset(nCD_b, -CD)

        adjT_sb = persist.tile([128, JT, R], f16)
        # tbl[:, jt, 0:8]=raw f2 (prelu bias), 8:16=b1=exp(f2-c2),
        # 16:24=b2=exp(.2 f2-c2); j on partitions
        tbl = persist.tile([128, JT, 24], f32)
        wh8 = whtp.tile([128, JT, NH], f8e4)   # Wh e4m3, j on partitions

        rowsd = dram.tile([3, H, R], f16)      # 0:f1 raw, 1:a1, 2:a2
        rd = dram.tile([2, R], f16)
        g1d = dram.tile([1, R], f16)
        a1Dd = dram.tile([1, R], f16)
        a2Dd = dram.tile([1, R], f16)
        g2in = dram.tile([1, R], f16)
        ccG = dram.tile([NC, 1, R], f16, addr_space="Shared")
        ccinW = dram.tile([R, HID], f8e4)
        ccW = dram.tile([NC, R, HID], f8e4, addr_space="Shared")

        # adj^T load early, batched 3D-AP DMAs on the gpsimd queue
        adjT_ap = ADJT[:]
        for lo in range(0, JT, 12):
            hi = min(lo + 12, JT)
            nc.gpsimd.dma_start(
                out=adjT_sb[:, lo:hi, :],
                in_=bass.AP(tensor=adjT_ap.tensor,
                            offset=adjT_ap.offset + lo * 128 * R,
                            ap=[[R, 128], [128 * R, hi - lo], [1, R]]))

        # --- phase-B SBUF pools opened FIRST so B elementwise overlaps A ---
        cwp = tc.tile_pool(name=pfx + "cw", bufs=1)
        cw = cwp.__enter__()
        bpools_cm = [tc.tile_pool(name=pfx + n, bufs=b) for n, b in
                     [("brep", 2), ("brr", 1), ("bt", 3), ("bp", 3),
                      ("belu", 1), ("bhc", 2)]]
        brep, brr, bt, bp, belu, bhc = [cm.__enter__() for cm in bpools_cm]

        wo_sb = cw.tile([128, H, HID], f16, tag="wo")
        for k8 in range(H):
            nc.sync.dma_start(out=wo_sb[:, k8, :],
                              in_=WO16[k8 * 128:(k8 + 1) * 128, :])
        aob = cw.tile([128, 2], f16, tag="aob")
        nc.sync.dma_start(out=aob, in_=AOB16[:])

        # ============ Phase A: projections + tables + Wh ============
        # Wa[k, 16]: cols 0..7 = per-head a1-projected W, 8..15 = a2
        af = tc.tile_pool(name=pfx + "af", bufs=1)
        afp = af.__enter__()
        wa16 = afp.tile([128, 3, 16], f16)
        with tc.tile_pool(name=pfx + "a0", bufs=1) as a0, \
             tc.tile_pool(name=pfx + "a0ps", bufs=1, space="PSUM") as a0ps:
            wt32 = a0.tile([128, H, KP], f32)
            nc.sync.dma_start(out=wt32, in_=WT32[:])
            a12_sb = a0.tile([128, H, 2], f32)
            nc.sync.dma_start(out=a12_sb, in_=A12[:])
            pwa = a0ps.tile([128, 3, 16], f32, tag="pwa")
            for c3 in range(3):
                for h in range(H):
                    nc.tensor.matmul(pwa[:, c3, h:h + 1],
                                     wt32[:, h, c3 * 128:(c3 + 1) * 128],
                                     a12_sb[:, h, 0:1], start=True, stop=True)
                    nc.tensor.matmul(pwa[:, c3, 8 + h:9 + h],
                                     wt32[:, h, c3 * 128:(c3 + 1) * 128],
                                     a12_sb[:, h, 1:2], start=True, stop=True)
            nc.any.tensor_copy(out=wa16, in_=pwa)

        # f2^T [8, NPAD] fp16 and Wh (e4m3) from ONE streamed read of x^T
        f2T8 = afp.tile([8, NPAD], f16)
        cp_eng = [nc.vector, nc.scalar]
        with tc.tile_pool(name=pfx + "a1", bufs=2) as a1, \
             tc.tile_pool(name=pfx + "a1w", bufs=1) as a1w, \
             tc.tile_pool(name=pfx + "a1ps", bufs=2, space="PSUM") as a1ps, \
             tc.tile_pool(name=pfx + "a1wps", bufs=3, space="PSUM") as a1wps:
            w16_sb = a1w.tile([128, 3, NH], f16)
            for c3 in range(3):
                nc.sync.dma_start(out=w16_sb[:, c3, :],
                                  in_=W16[c3 * 128:(c3 + 1) * 128, :])
            for ncol in range(0, NPAD, 512):
                w = min(512, NPAD - ncol)
                xt = a1.tile([128, 3, 512], f16, tag="xt")
                for c3 in range(3):
                    nc.sync.dma_start(
                        out=xt[:, c3, :w],
                        in_=xT16[c3 * 128:(c3 + 1) * 128, ncol:ncol + w])
                pf = a1ps.tile([8, 512], f32, tag="pf")
                for c3 in range(3):
                    nc.tensor.matmul(pf[:, :w], wa16[:, c3, 8:16],
                                     xt[:, c3, :w],
                                     start=(c3 == 0), stop=(c3 == 2))
                nc.any.tensor_copy(out=f2T8[:, ncol:ncol + w], in_=pf[:, :w])
                for sub in range(w // 128):
                    jt = (ncol + sub * 128) // 128
                    for half in range(2):
                        pw = a1wps.tile([128, 512], f32, tag="pw")
                        for c3 in range(3):
                            nc.tensor.matmul(
                                pw, xt[:, c3, sub * 128:(sub + 1) * 128],
                                w16_sb[:, c3, half * 512:(half + 1) * 512],
                                start=(c3 == 0), stop=(c3 == 2))
                        eng = cp_eng[(jt * 2 + half) % len(cp_eng)]
                        if eng is nc.scalar:
                            nc.scalar.activation(
                                out=wh8[:, jt, half * 512:(half + 1) * 512],
                                in_=pw, func=AF.Copy)
                        else:
                            eng.tensor_copy(
                                out=wh8[:, jt, half * 512:(half + 1) * 512],
                                in_=pw)

        # f rows for this core's rows -> raw f1 + a1 + a2 (fp16 rows in DRAM)
        with tc.tile_pool(name=pfx + "a2", bufs=1) as a2s, \
             tc.tile_pool(name=pfx + "a2ps", bufs=1, space="PSUM") as a2ps:
            xtl = a2s.tile([128, 3, R], f16)
            for c3 in range(3):
                nc.sync.dma_start(out=xtl[:, c3, :],
                                  in_=xTl16[c3 * 128:(c3 + 1) * 128, :])
            p1 = a2ps.tile([16, R], f32, tag="p1")
            for c3 in range(3):
                _mm_acc(nc, p1, wa16[:, c3, :], xtl[:, c3, :],
                        start=(c3 == 0), stop=(c3 == 2))
            f1row = a2s.tile([8, R], f16)
            nc.any.tensor_copy(out=f1row, in_=p1[0:8, :])
            nc.sync.dma_start(out=rowsd[0], in_=f1row)
            a1row = a2s.tile([8, R], f16)
            nc.scalar.activation(out=a1row, in_=p1[0:8, :], func=AF.Exp,
                                 bias=nC1_b[0:8], scale=1.0)
            nc.sync.dma_start(out=rowsd[1], in_=a1row)
            a2row = a2s.tile([8, R], f16)
            nc.scalar.activation(out=a2row, in_=p1[0:8, :], func=AF.Exp,
                                 bias=nC1_b[0:8], scale=ALPHA)
            nc.sync.dma_start(out=rowsd[2], in_=a2row)

        # b1/b2 rows from f2, then per-jt transposes -> table
        b1row8 = afp.tile([8, NPAD], f16)
        nc.scalar.activation(out=b1row8, in_=f2T8, func=AF.Exp,
                             bias=nC1_b[0:8], scale=1.0)
        b2row8 = afp.tile([8, NPAD], f16)
        nc.scalar.activation(out=b2row8, in_=f2T8, func=AF.Exp,
                             bias=nC1_b[0:8], scale=ALPHA)
        with tc.tile_pool(name=pfx + "a3ps", bufs=2, space="PSUM") as a3ps:
            for jt in range(JT):
                pt = a3ps.tile([128, 24], f16, tag="pt")
                sl = slice(jt * 128, (jt + 1) * 128)
                nc.tensor.transpose(pt[:, 0:8], f2T8[:, sl], ident16[:8, :8])
                nc.tensor.transpose(pt[:, 8:16], b1row8[:, sl],
                                    ident16[:8, :8])
                nc.tensor.transpose(pt[:, 16:24], b2row8[:, sl],
                                    ident16[:8, :8])
                nc.vector.tensor_copy(out=tbl[:, jt, :], in_=pt)
        af.__exit__(None, None, None)

        # ============ Phase B: layer-1 attention + fused Wh2 accum ============
        w2ps_cm = tc.tile_pool(name=pfx + "w2ps", bufs=1, space="PSUM")
        w2ps = w2ps_cm.__enter__()
        psW2 = w2ps.tile([128, R], f32, tag="psW2")

        with tc.tile_pool(name=pfx + "bps", bufs=2, space="PSUM") as bps, \
             tc.tile_pool(name=pfx + "brps", bufs=1, space="PSUM") as brps:
            for h in range(H):
                hsl = slice(h * 128, (h + 1) * 128)
                f1rep = brep.tile([128, R], f16, tag="f1rep")
                nc.sync.dma_start(out=f1rep,
                                  in_=_bcast_row(bass, rowsd[0, h:h + 1, :]))
                a1rep = brep.tile([128, R], f16, tag="a1rep")
                nc.sync.dma_start(out=a1rep,
                                  in_=_bcast_row(bass, rowsd[1, h:h + 1, :]))
                a2rep = brep.tile([128, R], f16, tag="a2rep")
                nc.sync.dma_start(out=a2rep,
                                  in_=_bcast_row(bass, rowsd[2, h:h + 1, :]))

                psA = bps.tile([128, R], f32, tag="psA")
                psR = brps.tile([1, R], f32, tag="psR")
                pair = None
                for jt in range(JT):
                    form = _form(jt)
                    if form == "S":
                        # ScalarE path: u = exp(lrelu(f1+f2) - SHIFT1)
                        e_t = bt.tile([128, R], f16, tag="t1")
                        nc.scalar.activation(out=e_t, in_=f1rep, func=AF.Prelu,
                                             bias=tbl[:, jt, h:h + 1],
                                             scale=1.0, alpha=ALPHA)
                        u_t = bt.tile([128, R], f16, tag="u")
                        nc.scalar.activation(out=u_t, in_=e_t, func=AF.Exp,
                                             bias=nSH1_b, scale=1.0)
                    else:
                        # separable path: u = max(a1*b1, a2*b2)
                        t1 = bt.tile([128, R], f16, tag="t1")
                        nc.vector.tensor_scalar_mul(t1, a1rep,
                                                    tbl[:, jt, 8 + h:9 + h])
                        t2 = bt.tile([128, R], f16, tag="t2")
                        t2eng = nc.gpsimd if form == "F" else nc.vector
                        t2eng.tensor_scalar_mul(t2, a2rep,
                                                tbl[:, jt, 16 + h:17 + h])
                        u_t = bt.tile([128, R], f16, tag="u")
                        nc.vector.tensor_tensor(out=u_t, in0=t1, in1=t2,
                                                op=ALU.max)
                    if jt % 2 == 0:
                        pair = bp.tile([128, 2, PR], f8e5, tag="p")
                    dst = pair[:, jt % 2, 0:R]
                    meng = nc.gpsimd if form == "G" else nc.vector
                    meng.tensor_tensor(out=dst, in0=u_t,
                                       in1=adjT_sb[:, jt, :],
                                       op=ALU.mult)
                    if jt % 2 == 1:
                        t = jt // 2
                        _mm_acc(nc, psA, wh8[:, 2 * t:2 * t + 2, hsl],
                                pair[:, :, 0:R], start=(t == 0), stop=False,
                                perf_mode=DR)
                        _mm_acc(nc, psR, ones8[:, :, 0:1], pair[:, :, 0:R],
                                start=(t == 0), stop=False, perf_mode=DR)
                    elif jt == JT - 1:
                        # odd tail: plain fp8 matmul on the half-filled pair
                        _mm_acc(nc, psA, wh8[:, jt, hsl], pair[:, 0, 0:R],
                                start=False, stop=True)
                        _mm_acc(nc, psR, ones8[:, 0, 0:1], pair[:, 0, 0:R],
                                start=False, stop=True)

                # normalize + (elu+1) -> hc chunk; psW2 += Wout_h^T @ hc
                lnr = belu.tile([1, R], f32, tag="lnr")
                nc.scalar.activation(out=lnr, in_=psR, func=AF.Ln)
                r16 = belu.tile([1, R], f16, tag="r16")
                nc.scalar.activation(out=r16, in_=lnr, func=AF.Exp,
                                     bias=0.0, scale=-1.0)
                nc.sync.dma_start(out=rd[0:1, :], in_=r16)
                rrep = brr.tile([128, R], f16, tag="rrep")
                nc.sync.dma_start(out=rrep, in_=_bcast_row(bass, rd[0:1, :]))
                v16 = bt.tile([128, R], f16, tag="t1")
                nc.vector.tensor_tensor(out=v16, in0=psA, in1=rrep,
                                        op=ALU.mult)
                neg_t = bt.tile([128, R], f16, tag="u")
                nc.vector.tensor_scalar_min(neg_t, v16, 0.0)
                en_t = bt.tile([128, R], f16, tag="t2")
                nc.scalar.activation(out=en_t, in_=neg_t, func=AF.Exp,
                                     bias=zero_b, scale=1.0)
                # hc = max(v,0) + exp(min(v,0)) = elu(v) + 1; the "-1" is
                # folded into the Wout column-sum correction below
                hc_t = bhc.tile([128, R], f16, tag="hc")
                nc.vector.scalar_tensor_tensor(out=hc_t, in0=v16, scalar=0.0,
                                               in1=en_t, op0=ALU.max,
                                               op1=ALU.add)
                _mm_acc(nc, psW2, wo_sb[:, h, :], hc_t,
                        start=(h == 0), stop=(h == H - 1))

        whtp_cm.__exit__(None, None, None)
        for cm in reversed(bpools_cm):
            cm.__exit__(None, None, None)

        # ============ Phase C: Wh2, g1/g2, AllGathers ============
        late_cm = tc.tile_pool(name=pfx + "late", bufs=1)
        late = late_cm.__enter__()
        g1rep = late.tile([128, R], f16)
        a1repD = late.tile([128, R], f16)
        a2repD = late.tile([128, R], f16)
        g2j = late.tile([128, JT], f16)
        g2j32 = late.tile([128, JT], f32)
        b1D = late.tile([128, JT], f32)
        b2D = late.tile([128, JT], f32)
        wh2j8 = late.tile([128, JT, HID], f8e4)

        with tc.tile_pool(name=pfx + "c1", bufs=2) as c1, \
             tc.tile_pool(name=pfx + "cps", bufs=2, space="PSUM") as cps:
            # Wout column sums (o on partitions) for the elu "-1" correction
            psScol = cps.tile([128, 1], f32, tag="psScol", bufs=1)
            ones16 = c1.tile([128, 1], f16, tag="ones16")
            nc.vector.memset(ones16, 1.0)
            for k8 in range(H):
                nc.tensor.matmul(psScol, wo_sb[:, k8, :], ones16,
                                 start=(k8 == 0), stop=(k8 == H - 1))
            scol = late.tile([128, 1], f32)
            nc.any.tensor_copy(out=scol, in_=psScol)
            wh2T16 = late.tile([128, R], f16)
            nc.vector.tensor_scalar_sub(wh2T16, psW2, scol)

            # g1/g2 (own rows); g2 gathered in a small parallel collective
            psG1 = cps.tile([1, R], f32, tag="psG", bufs=1)
            _mm_acc(nc, psG1, aob[:, 0:1], wh2T16, start=True, stop=True)
            g1row = late.tile([1, R], f16)
            nc.any.tensor_copy(out=g1row, in_=psG1)
            nc.sync.dma_start(out=g1d, in_=g1row)
            a1Drow = late.tile([1, R], f16)
            nc.scalar.activation(out=a1Drow, in_=psG1, func=AF.Exp,
                                 bias=nCD_b[0:1], scale=1.0)
            nc.sync.dma_start(out=a1Dd, in_=a1Drow)
            a2Drow = late.tile([1, R], f16)
            nc.scalar.activation(out=a2Drow, in_=psG1, func=AF.Exp,
                                 bias=nCD_b[0:1], scale=ALPHA)
            nc.sync.dma_start(out=a2Dd, in_=a2Drow)

            psG2 = cps.tile([1, R], f32, tag="psG", bufs=1)
            _mm_acc(nc, psG2, aob[:, 1:2], wh2T16, start=True, stop=True)
            g2row = late.tile([1, R], f16)
            nc.any.tensor_copy(out=g2row, in_=psG2)
            nc.sync.dma_start(out=g2in, in_=g2row)
            nc.gpsimd.collective_compute(
                "AllGather", mybir.AluOpType.bypass,
                replica_groups=[list(range(NC))],
                ins=[g2in.opt()], outs=[ccG.opt()])

            # layer-2 tables from the gathered g2 row (j on partitions)
            nc.vector.memset(g2j, 0.0)
            nc.sync.dma_start(
                out=g2j[:, 0:JT - 1],
                in_=_dram_ap(bass, ccG, 0, [[1, 128], [128, JT - 1]]))
            nc.sync.dma_start(
                out=g2j[:N - (JT - 1) * 128, JT - 1:JT],
                in_=_dram_ap(bass, ccG, (JT - 1) * 128,
                             [[1, N - (JT - 1) * 128], [1, 1]]))
            nc.vector.tensor_copy(out=g2j32, in_=g2j)
            nc.scalar.activation(out=b1D, in_=g2j, func=AF.Exp,
                                 bias=nCD_b, scale=1.0)
            nc.scalar.activation(out=b2D, in_=g2j, func=AF.Exp,
                                 bias=nCD_b, scale=ALPHA)
            nc.sync.dma_start(out=g1rep, in_=_bcast_row(bass, g1d[0:1, :]))
            nc.sync.dma_start(out=a1repD, in_=_bcast_row(bass, a1Dd[0:1, :]))
            nc.sync.dma_start(out=a2repD, in_=_bcast_row(bass, a2Dd[0:1, :]))

            # transpose Wh2^T locally -> row layout (e4m3), then big gather
            for it in range(6):
                w = min(128, R - it * 128)
                ptc = cps.tile([128, 128], f16, tag="ptc")
                nc.tensor.transpose(ptc[:w, :],
                                    wh2T16[:, it * 128:it * 128 + w], ident16)
                trs = c1.tile([128, 128], f8e4, tag="trs")
                nc.any.tensor_copy(out=trs[:w, :], in_=ptc[:w, :])
                nc.sync.dma_start(out=ccinW[it * 128:it * 128 + w, :],
                                  in_=trs[:w, :])
            nc.gpsimd.collective_compute(
                "AllGather", mybir.AluOpType.bypass,
                replica_groups=[list(range(NC))],
                ins=[ccinW.opt()], outs=[ccW.opt()])
        w2ps_cm.__exit__(None, None, None)

        # ============ Phase D: layer-2 attention ============
        # all elementwise + rowsums first (overlaps the big AllGather);
        # aggregation matmuls once wh2j8 lands
        with tc.tile_pool(name=pfx + "dt", bufs=3) as dt_, \
             tc.tile_pool(name=pfx + "dp", bufs=24) as dp, \
             tc.tile_pool(name=pfx + "dfin", bufs=1) as dfin, \
             tc.tile_pool(name=pfx + "dout", bufs=2) as dout, \
             tc.tile_pool(name=pfx + "dps", bufs=1, space="PSUM") as dps, \
             tc.tile_pool(name=pfx + "dops", bufs=2, space="PSUM") as dops:
            psA2 = dps.tile([128, R], f32, tag="psA2")
            psR2 = dps.tile([1, R], f32, tag="psR2")
            pairs = []
            pair = None
            for jt in range(JT):
                form = _form(jt)
                if form == "S":
                    e_t = dt_.tile([128, R], f16, tag="t1")
                    nc.scalar.activation(out=e_t, in_=g1rep, func=AF.Prelu,
                                         bias=g2j32[:, jt:jt + 1],
                                         scale=1.0, alpha=ALPHA)
                    u2 = dt_.tile([128, R], f16, tag="u2")
                    nc.scalar.activation(out=u2, in_=e_t, func=AF.Exp,
                                         bias=nSH2_b, scale=1.0)
                else:
                    t1 = dt_.tile([128, R], f16, tag="t1")
                    nc.vector.tensor_scalar_mul(t1, a1repD, b1D[:, jt:jt + 1])
                    t2 = dt_.tile([128, R], f16, tag="t2")
                    t2eng = nc.gpsimd if form == "F" else nc.vector
                    t2eng.tensor_scalar_mul(t2, a2repD, b2D[:, jt:jt + 1])
                    u2 = dt_.tile([128, R], f16, tag="u2")
                    nc.vector.tensor_tensor(out=u2, in0=t1, in1=t2, op=ALU.max)
                if jt % 2 == 0:
                    pair = dp.tile([128, 2, PR], f8e5, tag="p2")
                    pairs.append(pair)
                meng = nc.gpsimd if form == "G" else nc.vector
                meng.tensor_tensor(out=pair[:, jt % 2, 0:R], in0=u2,
                                   in1=adjT_sb[:, jt, :], op=ALU.mult)
                if jt % 2 == 1:
                    t = jt // 2
                    _mm_acc(nc, psR2, ones8[:, :, 0:1], pair[:, :, 0:R],
                            start=(t == 0), stop=False, perf_mode=DR)
                elif jt == JT - 1:
                    _mm_acc(nc, psR2, ones8[:, 0, 0:1], pair[:, 0, 0:R],
                            start=False, stop=True)

            # load the gathered Wh2 (e4m3, j on partitions) and aggregate
            nc.vector.memset(wh2j8[:, JT - 1, :], 0.0)
            nc.sync.dma_start(
                out=wh2j8[:, 0:JT - 1, :],
                in_=_dram_ap(bass, ccW, 0,
                             [[HID, 128], [128 * HID, JT - 1], [1, HID]]))
            nc.sync.dma_start(
                out=wh2j8[:N - (JT - 1) * 128, JT - 1, :],
                in_=_dram_ap(bass, ccW, (JT - 1) * 128 * HID,
                             [[HID, N - (JT - 1) * 128], [1, HID]]))
            for t in range(JT // 2):
                _mm_acc(nc, psA2, wh2j8[:, 2 * t:2 * t + 2, :],
                        pairs[t][:, :, 0:R], start=(t == 0), stop=False,
                        perf_mode=DR)
            _mm_acc(nc, psA2, wh2j8[:, JT - 1, :], pairs[-1][:, 0, 0:R],
                    start=False, stop=True)

            lnr2 = dfin.tile([1, R], f32, tag="lnr2")
            nc.scalar.activation(out=lnr2, in_=psR2, func=AF.Ln)
            r216 = dfin.tile([1, R], f16, tag="r216")
            nc.scalar.activation(out=r216, in_=lnr2, func=AF.Exp,
                                 bias=0.0, scale=-1.0)
            nc.sync.dma_start(out=rd[1:2, :], in_=r216)
            r2rep = dfin.tile([128, R], f16, tag="r2rep")
            nc.sync.dma_start(out=r2rep, in_=_bcast_row(bass, rd[1:2, :]))
            o_t = dfin.tile([128, R], f32, tag="o")
            nc.vector.tensor_tensor(out=o_t, in0=psA2, in1=r2rep,
                                    op=ALU.mult)

            # transpose back to row layout and write out
            for it in range(6):
                w = min(128, R - it * 128)
                po = dops.tile([128, 128], f32, tag="po")
                nc.tensor.transpose(po[:w, :],
                                    o_t[:, it * 128:it * 128 + w], ident32)
                orow = dout.tile([128, 128], f32, tag="orow")
                nc.any.tensor_copy(out=orow[:w, :], in_=po[:w, :])
                nc.sync.dma_start(out=OUT[it * 128:it * 128 + w, :],
                                  in_=orow[:w, :])
        late_cm.__exit__(None, None, None)
        cwp.__exit__(None, None, None)


def _host_prep(x, adj, W_heads, a_heads, W_out, a_out):
    """Per-core input maps. Layout/pad/cast only -- no model math."""
    xT = np.zeros((KP, NPAD), np.float16)
    xT[:F_IN, :N] = x.T.astype(np.float16)
    W16 = np.zeros((KP, NH), np.float16)
    W16[:F_IN] = W_heads.transpose(1, 0, 2).reshape(F_IN, NH).astype(np.float16)
    WT32 = np.zeros((HID, H, KP), np.float32)
    WT32[:, :, :F_IN] = W_heads.transpose(2, 0, 1)
    a12 = np.stack([a_heads[:, :HID, 0], a_heads[:, HID:, 0]], axis=2)
    a12 = np.ascontiguousarray(a12.transpose(1, 0, 2)).astype(np.float32)
    aob = np.concatenate([a_out[:HID], a_out[HID:]], axis=1).astype(np.float16)
    Wout16 = W_out.astype(np.float16)

    in_maps = []
    for c in range(NC):
        rows = slice(c * R, (c + 1) * R)
        adjT = np.zeros((NPAD, R), np.float16)
        adjT[:N, :] = adj[rows].T
        xTl = np.zeros((KP, R), np.float16)
        xTl[:F_IN] = x[rows].T.astype(np.float16)
        in_maps.append({
            "xT16": xT, "xTl16": xTl, "W16": W16, "WT32": WT32, "a12": a12,
            "aob16": aob, "Wout16": Wout16,
            "adjT": np.ascontiguousarray(adjT),
        })
    return in_maps


def run(inputs, trace=False, **kw):
    from concourse.bass_utils import run_bass_kernel_spmd
    if "nc" not in _CACHE:
        _CACHE["nc"] = _build()
    nc = _CACHE["nc"]
    in_maps = _host_prep(**inputs)
    res = run_bass_kernel_spmd(nc, in_maps, core_ids=list(range(NC)),
                               trace=trace, **kw)
    out = np.concatenate([res.results[c]["out"] for c in range(NC)], axis=0)
    return out, res


def kernel(x, adj, W_heads, a_heads, W_out, a_out):
    out, _ = run(dict(x=np.asarray(x), adj=np.asarray(adj),
                      W_heads=np.asarray(W_heads), a_heads=np.asarray(a_heads),
                      W_out=np.asarray(W_out), a_out=np.asarray(a_out)))
    return out


# revision 17
# speedup vs baseline: 1.7831x; 1.7831x over previous
"""GAT (2-layer dense graph attention) on 8 Trainium2 NeuronCores.

Sharding: nodes (rows) split 8 ways, 750 rows/core, column-on-partition
("transposed") layout so attention probabilities feed TensorE directly.

v3 vs the 845us baseline:
  - BOTH branches of exp(lrelu(s) - T) are separable outer products:
      exp(s - T)   = exp(f1_i - c1) * exp(f2_j - c2)
      exp(.2s - T) = exp(.2f1_i - c1) * exp(.2f2_j - c2)   (c1 + c2 = T)
    so u = max(a1*b1, a2*b2) needs NO per-tile exp. Shifts c1=c2=4 (layer 1),
    1.5 (layer 2) sized from the reference logit ranges (|f|<6.7, |g|<1.3) so
    every fp16 intermediate is in range (factors <= e^1.8, 1/rowsum <= e^10.7).
  - attention aggregation + rowsum matmuls run in fp8 DoubleRow perf mode
    (2 j-tiles per PE pass): p in e5m2, Wh/Wh2 stationaries in e4m3.
    Measured end-to-end error of this mix on the reference inputs: 6.4e-3.
  - per-tile elementwise work round-robins across DVE (tensor_scalar +
    max + mask), ScalarE (prelu+exp form), GpSimd (max or t2+max, fp16 only)
    to balance the three engines; psum->sbuf copies also rotate engines.
  - phase-B elementwise SBUF pools are allocated before phase A so the B
    work overlaps A's PE-bound projection phase (no pool-boundary stall).
  - hc stays in SBUF feeding the Wh2 accumulation per head; the ELU "-1" is
    folded into a rank-1 Wout column-sum correction on the psW2 copy-out.
  - g2 is computed from local Wh2 rows and gathered in a small collective
    BEFORE the big (fp8) Wh2 AllGather; all layer-2 elementwise + rowsums
    overlap the big gather, aggregation runs once it lands.
"""

import sys
import numpy as np

sys.path.insert(0, "/opt/trn_rl_repo")

N = 6000
F_IN = 300
HID = 128
H = 8
NC = 8          # cores
R = 750         # rows per core
JT = 47         # j tiles of 128 (6016 padded)
NPAD = JT * 128  # 6016
KP = 384        # padded F_IN (3 chunks of 128)
NH = H * HID    # 1024
ALPHA = 0.2
C1 = 4.0        # layer-1 exp shift split (c1 + c2 = SHIFT1)
SHIFT1 = 8.0
CD = 1.5        # layer-2 shift split
SHIFT2 = 3.0
PR = 768        # padded pair-tile row length (stride % 16 == 0)

_CACHE = {}


def _form(jt):
    """Pair-grouped engine/precision split. S-pairs: ScalarE prelu+exp,
    DVE fp16 mask, fp16 matmuls. G-pairs: DVE separable u, GpSimd fp8e5
    mask, fp8 DoubleRow matmuls."""
    return "S" if (jt // 2) % 2 == 0 else "G"


def _sidx(jt):
    """Index of an S-form jt within the compacted fp16 weight table."""
    return (jt // 4) * 2 + (jt % 4)


def _gidx(jt):
    """Index of a G-form jt within the compacted fp8 weight table."""
    return 22 if jt == JT - 1 else (jt // 4) * 2 + (jt % 4 - 2)


def _mm_acc(nc, psum, lhsT, rhs, start, stop, width=512, perf_mode=None):
    n = rhs.shape[-1]
    for lo in range(0, n, width):
        hi = min(lo + width, n)
        nc.tensor.matmul(psum[:, lo:hi], lhsT, rhs[..., lo:hi],
                         start=start, stop=stop, perf_mode=perf_mode)


def _bcast_row(bass, row_ap, parts=128):
    """AP that reads a [1, n] DRAM row replicated across `parts` partitions."""
    return bass.AP(tensor=row_ap.tensor, offset=row_ap.offset,
                   ap=[[0, parts]] + [list(d) for d in row_ap.ap[1:]])


def _dram_ap(bass, tile_ap, offset_elems, dims):
    """Manual AP over a DRAM tile at an element offset with [stride,count] dims."""
    return bass.AP(tensor=tile_ap.tensor, offset=tile_ap.offset + offset_elems,
                   ap=[list(d) for d in dims])


def _build(reps=1):
    import concourse.bass as bass
    import concourse.tile as tile
    import concourse.tile_utils as tile_utils
    from concourse import bacc, mybir
    from concourse.masks import make_identity

    tile_utils.max_sbuf_usage = 206 * 1024

    f32, f16 = mybir.dt.float32, mybir.dt.float16
    f8e4, f8e5 = mybir.dt.float8e4, mybir.dt.float8e5
    AF = mybir.ActivationFunctionType
    ALU = mybir.AluOpType

    nc = bacc.Bacc("TRN2", target_bir_lowering=False, debug=False,
                   enable_asserts=False, num_devices=NC)

    xT16 = nc.dram_tensor("xT16", [KP, NPAD], f16, kind="ExternalInput")
    xTl16 = nc.dram_tensor("xTl16", [KP, R], f16, kind="ExternalInput")
    W16 = nc.dram_tensor("W16", [KP, NH], f16, kind="ExternalInput")
    WT32 = nc.dram_tensor("WT32", [HID, H, KP], f16, kind="ExternalInput")
    A12 = nc.dram_tensor("a12", [HID, H, 2], f16, kind="ExternalInput")
    AOB16 = nc.dram_tensor("aob16", [HID, 2], f16, kind="ExternalInput")
    WO16 = nc.dram_tensor("Wout16", [NH, HID], f16, kind="ExternalInput")
    ADJT = nc.dram_tensor("adjT", [NPAD, R], f16, kind="ExternalInput")
    OUT = nc.dram_tensor("out", [R, HID], f32, kind="ExternalOutput")

    with tile.TileContext(nc) as tc:
        for rep in range(reps):
            _body(nc, tc, bass, tile, mybir, f32, f16, f8e4, f8e5, AF, ALU,
                  make_identity, xT16, xTl16, W16, WT32, A12, AOB16, WO16,
                  ADJT, OUT, pfx=f"r{rep}_" if reps > 1 else "")
    nc.compile()
    return nc


def _body(nc, tc, bass, tile, mybir, f32, f16, f8e4, f8e5, AF, ALU,
          make_identity, xT16, xTl16, W16, WT32, A12, AOB16, WO16, ADJT, OUT,
          pfx=""):
    DR = mybir.MatmulPerfMode.DoubleRow
    whtp_cm = tc.tile_pool(name=pfx + "whtp", bufs=1, side="right")
    whtp = whtp_cm.__enter__()
    with tc.tile_pool(name=pfx + "persist", bufs=1) as persist, \
         tc.tile_pool(name=pfx + "dram", bufs=1, space="DRAM") as dram:

        ident32 = persist.tile([128, 128], f32)
        make_identity(nc, ident32)
        ident16 = persist.tile([128, 128], f16)
        nc.vector.tensor_copy(out=ident16, in_=ident32)
        ones8 = persist.tile([128, 2, 16], f8e4)
        nc.vector.memset(ones8, 1.0)
        ones16 = persist.tile([128, 1], f16)
        nc.vector.memset(ones16, 1.0)
        zero_b = persist.tile([128, 1], f32)
        nc.vector.memset(zero_b, 0.0)
        nSH1_b = persist.tile([128, 1], f32)
        nc.vector.memset(nSH1_b, -SHIFT1)
        nSH2_b = persist.tile([128, 1], f32)
        nc.vector.memset(nSH2_b, -SHIFT2)
        nC1_b = persist.tile([128, 1], f32)
        nc.vector.memset(nC1_b, -C1)
        nCD_b = persist.tile([128, 1], f32)
        nc.vector.memset(nCD_b, -CD)

        adjT_sb = persist.tile([128, JT, R], f16)
        # tbl[:, jt, 0:8]=raw f2 (prelu bias), 8:16=b1=exp(f2-c2),
        # 16:24=b2=exp(.2 f2-c2); j on partitions
        tbl = persist.tile([128, JT, 24], f32)
        wh16S = whtp.tile([128, 24, NH], f16)  # Wh fp16, S-form j-tiles
        wh8G = whtp.tile([128, 23, NH], f8e4)  # Wh e4m3, G-form j-tiles

        rowsd = dram.tile([3, H, R], f16)      # 0:f1 raw, 1:a1, 2:a2
        rd = dram.tile([2, R], f16)
        g1d = dram.tile([1, R], f16)
        a1Dd = dram.tile([1, R], f16)
        a2Dd = dram.tile([1, R], f16)
        g2in = dram.tile([1, R], f16)
        ccG = dram.tile([NC, 1, R], f16, addr_space="Shared")
        ccinW = dram.tile([R, HID], f8e4)
        ccW = dram.tile([NC, R, HID], f8e4, addr_space="Shared")

        # adj^T load early, batched 3D-AP DMAs on the gpsimd queue
        adjT_ap = ADJT[:]
        for lo in range(0, JT, 12):
            hi = min(lo + 12, JT)
            nc.gpsimd.dma_start(
                out=adjT_sb[:, lo:hi, :],
                in_=bass.AP(tensor=adjT_ap.tensor,
                            offset=adjT_ap.offset + lo * 128 * R,
                            ap=[[R, 128], [128 * R, hi - lo], [1, R]]))

        # --- phase-B SBUF pools opened FIRST so B elementwise overlaps A ---
        cwp = tc.tile_pool(name=pfx + "cw", bufs=1)
        cw = cwp.__enter__()
        bpools_cm = [tc.tile_pool(name=pfx + n, bufs=b) for n, b in
                     [("brep", 2), ("brr", 1), ("bt", 2), ("bp", 3),
                      ("belu", 1), ("bhc", 2)]]
        brep, brr, bt, bp, belu, bhc = [cm.__enter__() for cm in bpools_cm]

        wo_sb = cw.tile([128, H, HID], f16, tag="wo")
        for k8 in range(H):
            nc.sync.dma_start(out=wo_sb[:, k8, :],
                              in_=WO16[k8 * 128:(k8 + 1) * 128, :])
        aob = cw.tile([128, 2], f16, tag="aob")
        nc.sync.dma_start(out=aob, in_=AOB16[:])

        # ============ Phase A: projections + tables + Wh ============
        # Wa[k, 16]: cols 0..7 = per-head a1-projected W, 8..15 = a2
        af = tc.tile_pool(name=pfx + "af", bufs=1)
        afp = af.__enter__()
        wa16 = afp.tile([128, 3, 16], f16)
        with tc.tile_pool(name=pfx + "a0", bufs=1) as a0, \
             tc.tile_pool(name=pfx + "a0ps", bufs=1, space="PSUM") as a0ps:
            wt32 = a0.tile([128, H, KP], f16)
            nc.sync.dma_start(out=wt32, in_=WT32[:])
            a12_sb = a0.tile([128, H, 2], f16)
            nc.sync.dma_start(out=a12_sb, in_=A12[:])
            pwa = a0ps.tile([128, 3, 16], f32, tag="pwa")
            for c3 in range(3):
                for h in range(H):
                    nc.tensor.matmul(pwa[:, c3, h:h + 1],
                                     wt32[:, h, c3 * 128:(c3 + 1) * 128],
                                     a12_sb[:, h, 0:1], start=True, stop=True)
                    nc.tensor.matmul(pwa[:, c3, 8 + h:9 + h],
                                     wt32[:, h, c3 * 128:(c3 + 1) * 128],
                                     a12_sb[:, h, 1:2], start=True, stop=True)
            nc.any.tensor_copy(out=wa16, in_=pwa)

        # f2^T [8, NPAD] fp16 and Wh (e4m3) from ONE streamed read of x^T
        f2T8 = afp.tile([8, NPAD], f16)
        cp_eng = [nc.vector, nc.scalar]
        with tc.tile_pool(name=pfx + "a1", bufs=2) as a1, \
             tc.tile_pool(name=pfx + "a1w", bufs=1) as a1w, \
             tc.tile_pool(name=pfx + "a1ps", bufs=2, space="PSUM") as a1ps, \
             tc.tile_pool(name=pfx + "a1wps", bufs=3, space="PSUM") as a1wps:
            w16_sb = a1w.tile([128, 3, NH], f16)
            for c3 in range(3):
                nc.sync.dma_start(out=w16_sb[:, c3, :],
                                  in_=W16[c3 * 128:(c3 + 1) * 128, :])
            for ncol in range(0, NPAD, 512):
                w = min(512, NPAD - ncol)
                xt = a1.tile([128, 3, 512], f16, tag="xt")
                for c3 in range(3):
                    nc.sync.dma_start(
                        out=xt[:, c3, :w],
                        in_=xT16[c3 * 128:(c3 + 1) * 128, ncol:ncol + w])
                pf = a1ps.tile([8, 512], f32, tag="pf")
                for c3 in range(3):
                    nc.tensor.matmul(pf[:, :w], wa16[:, c3, 8:16],
                                     xt[:, c3, :w],
                                     start=(c3 == 0), stop=(c3 == 2))
                nc.any.tensor_copy(out=f2T8[:, ncol:ncol + w], in_=pf[:, :w])
                for sub in range(w // 128):
                    jt = (ncol + sub * 128) // 128
                    for half in range(2):
                        pw = a1wps.tile([128, 512], f32, tag="pw")
                        for c3 in range(3):
                            nc.tensor.matmul(
                                pw, xt[:, c3, sub * 128:(sub + 1) * 128],
                                w16_sb[:, c3, half * 512:(half + 1) * 512],
                                start=(c3 == 0), stop=(c3 == 2))
                        if _form(jt) == "S":
                            wdst = wh16S[:, _sidx(jt),
                                         half * 512:(half + 1) * 512]
                        else:
                            wdst = wh8G[:, _gidx(jt),
                                        half * 512:(half + 1) * 512]
                        eng = cp_eng[(jt * 2 + half) % len(cp_eng)]
                        if eng is nc.scalar:
                            nc.scalar.activation(out=wdst, in_=pw,
                                                 func=AF.Copy)
                        else:
                            eng.tensor_copy(out=wdst, in_=pw)

        # f rows for this core's rows -> raw f1 + a1 + a2 (fp16 rows in DRAM)
        with tc.tile_pool(name=pfx + "a2", bufs=1) as a2s, \
             tc.tile_pool(name=pfx + "a2ps", bufs=1, space="PSUM") as a2ps:
            xtl = a2s.tile([128, 3, R], f16)
            for c3 in range(3):
                nc.sync.dma_start(out=xtl[:, c3, :],
                                  in_=xTl16[c3 * 128:(c3 + 1) * 128, :])
            p1 = a2ps.tile([16, R], f32, tag="p1")
            for c3 in range(3):
                _mm_acc(nc, p1, wa16[:, c3, :], xtl[:, c3, :],
                        start=(c3 == 0), stop=(c3 == 2))
            f1row = a2s.tile([8, R], f16)
            nc.any.tensor_copy(out=f1row, in_=p1[0:8, :])
            nc.sync.dma_start(out=rowsd[0], in_=f1row)
            a1row = a2s.tile([8, R], f16)
            nc.scalar.activation(out=a1row, in_=p1[0:8, :], func=AF.Exp,
                                 bias=nC1_b[0:8], scale=1.0)
            nc.sync.dma_start(out=rowsd[1], in_=a1row)
            a2row = a2s.tile([8, R], f16)
            nc.scalar.activation(out=a2row, in_=p1[0:8, :], func=AF.Exp,
                                 bias=nC1_b[0:8], scale=ALPHA)
            nc.sync.dma_start(out=rowsd[2], in_=a2row)

        # transpose raw f2 per jt, then build b1/b2 via Exp on the
        # transposed [128, 8] psum directly into the table
        with tc.tile_pool(name=pfx + "a3ps", bufs=2, space="PSUM") as a3ps:
            for jt in range(JT):
                pt = a3ps.tile([128, 8], f16, tag="pt")
                sl = slice(jt * 128, (jt + 1) * 128)
                nc.tensor.transpose(pt, f2T8[:, sl], ident16[:8, :8])
                nc.vector.tensor_copy(out=tbl[:, jt, 0:8], in_=pt)
                nc.scalar.activation(out=tbl[:, jt, 8:16], in_=pt, func=AF.Exp,
                                     bias=nC1_b, scale=1.0)
                nc.scalar.activation(out=tbl[:, jt, 16:24], in_=pt,
                                     func=AF.Exp, bias=nC1_b, scale=ALPHA)
        af.__exit__(None, None, None)

        # ============ Phase B: layer-1 attention + fused Wh2 accum ============
        w2ps_cm = tc.tile_pool(name=pfx + "w2ps", bufs=1, space="PSUM")
        w2ps = w2ps_cm.__enter__()
        psW2 = w2ps.tile([128, R], f32, tag="psW2")

        with tc.tile_pool(name=pfx + "bps", bufs=2, space="PSUM") as bps, \
             tc.tile_pool(name=pfx + "brps", bufs=1, space="PSUM") as brps:
            for h in range(H):
                hsl = slice(h * 128, (h + 1) * 128)
                f1rep = brep.tile([128, R], f16, tag="f1rep")
                nc.sync.dma_start(out=f1rep,
                                  in_=_bcast_row(bass, rowsd[0, h:h + 1, :]))
                a1rep = brep.tile([128, R], f16, tag="a1rep")
                nc.sync.dma_start(out=a1rep,
                                  in_=_bcast_row(bass, rowsd[1, h:h + 1, :]))
                a2rep = brep.tile([128, R], f16, tag="a2rep")
                nc.sync.dma_start(out=a2rep,
                                  in_=_bcast_row(bass, rowsd[2, h:h + 1, :]))

                psA = bps.tile([128, R], f32, tag="psA")
                psR = brps.tile([1, R], f32, tag="psR")
                pair = None
                for jt in range(JT):
                    form = _form(jt)
                    if form == "S":
                        # ScalarE path: u = exp(lrelu(f1+f2) - SHIFT1)
                        e_t = bt.tile([128, R], f16, tag="t1")
                        nc.scalar.activation(out=e_t, in_=f1rep, func=AF.Prelu,
                                             bias=tbl[:, jt, h:h + 1],
                                             scale=1.0, alpha=ALPHA)
                        u_t = bt.tile([128, R], f16, tag="u")
                        nc.scalar.activation(out=u_t, in_=e_t, func=AF.Exp,
                                             bias=nSH1_b, scale=1.0)
                        # fp16 path: DVE mask, per-jt fp16 matmuls
                        p16 = bp.tile([128, R], f16, tag="p16", bufs=2)
                        nc.vector.tensor_tensor(out=p16, in0=u_t,
                                                in1=adjT_sb[:, jt, :],
                                                op=ALU.mult)
                        _mm_acc(nc, psA, wh16S[:, _sidx(jt), hsl], p16,
                                start=(jt == 0), stop=False)
                        _mm_acc(nc, psR, ones16, p16,
                                start=(jt == 0), stop=False)
                    else:
                        # separable path: u = max(a1*b1, a2*b2); GpSimd
                        # fp8e5 mask; fp8 DoubleRow matmuls per pair
                        t1 = bt.tile([128, R], f16, tag="t1")
                        nc.vector.tensor_scalar_mul(t1, a1rep,
                                                    tbl[:, jt, 8 + h:9 + h])
                        t2 = bt.tile([128, R], f16, tag="t2")
                        nc.vector.tensor_scalar_mul(t2, a2rep,
                                                    tbl[:, jt, 16 + h:17 + h])
                        u_t = bt.tile([128, R], f16, tag="u")
                        nc.vector.tensor_tensor(out=u_t, in0=t1, in1=t2,
                                                op=ALU.max)
                        if jt % 2 == 0:
                            pair = bp.tile([128, 2, PR], f8e5, tag="p")
                        nc.gpsimd.tensor_tensor(out=pair[:, jt % 2, 0:R],
                                                in0=u_t,
                                                in1=adjT_sb[:, jt, :],
                                                op=ALU.mult)
                        gi = _gidx(jt)
                        if jt % 2 == 1:
                            _mm_acc(nc, psA, wh8G[:, gi - 1:gi + 1, hsl],
                                    pair[:, :, 0:R], start=False, stop=False,
                                    perf_mode=DR)
                            _mm_acc(nc, psR, ones8[:, :, 0:1],
                                    pair[:, :, 0:R], start=False, stop=False,
                                    perf_mode=DR)
                        elif jt == JT - 1:
                            _mm_acc(nc, psA, wh8G[:, gi, hsl], pair[:, 0, 0:R],
                                    start=False, stop=True)
                            _mm_acc(nc, psR, ones8[:, 0, 0:1], pair[:, 0, 0:R],
                                    start=False, stop=True)

                # normalize + (elu+1) -> hc chunk; psW2 += Wout_h^T @ hc
                lnr = belu.tile([1, R], f32, tag="lnr")
                nc.scalar.activation(out=lnr, in_=psR, func=AF.Ln)
                r16 = belu.tile([1, R], f16, tag="r16")
                nc.scalar.activation(out=r16, in_=lnr, func=AF.Exp,
                                     bias=0.0, scale=-1.0)
                nc.sync.dma_start(out=rd[0:1, :], in_=r16)
                rrep = brr.tile([128, R], f16, tag="rrep")
                nc.sync.dma_start(out=rrep, in_=_bcast_row(bass, rd[0:1, :]))
                v16 = bt.tile([128, R], f16, tag="t1")
                nc.vector.tensor_tensor(out=v16, in0=psA, in1=rrep,
                                        op=ALU.mult)
                neg_t = bt.tile([128, R], f16, tag="u")
                nc.vector.tensor_scalar_min(neg_t, v16, 0.0)
                en_t = bt.tile([128, R], f16, tag="t2")
                nc.scalar.activation(out=en_t, in_=neg_t, func=AF.Exp,
                                     bias=zero_b, scale=1.0)
                # hc = max(v,0) + exp(min(v,0)) = elu(v) + 1; the "-1" is
                # folded into the Wout column-sum correction below
                hc_t = bhc.tile([128, R], f16, tag="hc")
                nc.vector.scalar_tensor_tensor(out=hc_t, in0=v16, scalar=0.0,
                                               in1=en_t, op0=ALU.max,
                                               op1=ALU.add)
                _mm_acc(nc, psW2, wo_sb[:, h, :], hc_t,
                        start=(h == 0), stop=(h == H - 1))

        whtp_cm.__exit__(None, None, None)
        for cm in reversed(bpools_cm):
            cm.__exit__(None, None, None)

        # ============ Phase C: Wh2, g1/g2, AllGathers ============
        late_cm = tc.tile_pool(name=pfx + "late", bufs=1)
        late = late_cm.__enter__()
        g1rep = late.tile([128, R], f16)
        a1repD = late.tile([128, R], f16)
        a2repD = late.tile([128, R], f16)
        g2j = late.tile([128, JT], f16)
        g2j32 = late.tile([128, JT], f32)
        b1D = late.tile([128, JT], f32)
        b2D = late.tile([128, JT], f32)
        wh2j8 = late.tile([128, JT, HID], f8e4)

        with tc.tile_pool(name=pfx + "c1", bufs=2) as c1, \
             tc.tile_pool(name=pfx + "cps", bufs=2, space="PSUM") as cps:
            # Wout column sums (o on partitions) for the elu "-1" correction
            psScol = cps.tile([128, 1], f32, tag="psScol", bufs=1)
            for k8 in range(H):
                nc.tensor.matmul(psScol, wo_sb[:, k8, :], ones16,
                                 start=(k8 == 0), stop=(k8 == H - 1))
            scol = late.tile([128, 1], f32)
            nc.any.tensor_copy(out=scol, in_=psScol)
            wh2T16 = late.tile([128, R], f16)
            nc.vector.tensor_scalar_sub(wh2T16, psW2, scol)

            # g1/g2 (own rows); g2 gathered in a small parallel collective
            psG1 = cps.tile([1, R], f32, tag="psG", bufs=1)
            _mm_acc(nc, psG1, aob[:, 0:1], wh2T16, start=True, stop=True)
            g1row = late.tile([1, R], f16)
            nc.any.tensor_copy(out=g1row, in_=psG1)
            nc.sync.dma_start(out=g1d, in_=g1row)
            a1Drow = late.tile([1, R], f16)
            nc.scalar.activation(out=a1Drow, in_=psG1, func=AF.Exp,
                                 bias=nCD_b[0:1], scale=1.0)
            nc.sync.dma_start(out=a1Dd, in_=a1Drow)
            a2Drow = late.tile([1, R], f16)
            nc.scalar.activation(out=a2Drow, in_=psG1, func=AF.Exp,
                                 bias=nCD_b[0:1], scale=ALPHA)
            nc.sync.dma_start(out=a2Dd, in_=a2Drow)

            psG2 = cps.tile([1, R], f32, tag="psG", bufs=1)
            _mm_acc(nc, psG2, aob[:, 1:2], wh2T16, start=True, stop=True)
            g2row = late.tile([1, R], f16)
            nc.any.tensor_copy(out=g2row, in_=psG2)
            nc.sync.dma_start(out=g2in, in_=g2row)
            nc.gpsimd.collective_compute(
                "AllGather", mybir.AluOpType.bypass,
                replica_groups=[list(range(NC))],
                ins=[g2in.opt()], outs=[ccG.opt()])

            # layer-2 tables from the gathered g2 row (j on partitions)
            nc.vector.memset(g2j, 0.0)
            nc.sync.dma_start(
                out=g2j[:, 0:JT - 1],
                in_=_dram_ap(bass, ccG, 0, [[1, 128], [128, JT - 1]]))
            nc.sync.dma_start(
                out=g2j[:N - (JT - 1) * 128, JT - 1:JT],
                in_=_dram_ap(bass, ccG, (JT - 1) * 128,
                             [[1, N - (JT - 1) * 128], [1, 1]]))
            nc.vector.tensor_copy(out=g2j32, in_=g2j)
            nc.scalar.activation(out=b1D, in_=g2j, func=AF.Exp,
                                 bias=nCD_b, scale=1.0)
            nc.scalar.activation(out=b2D, in_=g2j, func=AF.Exp,
                                 bias=nCD_b, scale=ALPHA)
            nc.sync.dma_start(out=g1rep, in_=_bcast_row(bass, g1d[0:1, :]))
            nc.sync.dma_start(out=a1repD, in_=_bcast_row(bass, a1Dd[0:1, :]))
            nc.sync.dma_start(out=a2repD, in_=_bcast_row(bass, a2Dd[0:1, :]))

            # transpose Wh2^T locally -> row layout (e4m3), then big gather
            for it in range(6):
                w = min(128, R - it * 128)
                ptc = cps.tile([128, 128], f16, tag="ptc")
                nc.tensor.transpose(ptc[:w, :],
                                    wh2T16[:, it * 128:it * 128 + w], ident16)
                trs = c1.tile([128, 128], f8e4, tag="trs")
                nc.any.tensor_copy(out=trs[:w, :], in_=ptc[:w, :])
                nc.sync.dma_start(out=ccinW[it * 128:it * 128 + w, :],
                                  in_=trs[:w, :])
            nc.gpsimd.collective_compute(
                "AllGather", mybir.AluOpType.bypass,
                replica_groups=[list(range(NC))],
                ins=[ccinW.opt()], outs=[ccW.opt()])
        w2ps_cm.__exit__(None, None, None)

        # ============ Phase D: layer-2 attention ============
        # all elementwise + rowsums first (overlaps the big AllGather);
        # aggregation matmuls once wh2j8 lands
        with tc.tile_pool(name=pfx + "dt", bufs=3) as dt_, \
             tc.tile_pool(name=pfx + "dp", bufs=24) as dp, \
             tc.tile_pool(name=pfx + "dfin", bufs=1) as dfin, \
             tc.tile_pool(name=pfx + "dout", bufs=2) as dout, \
             tc.tile_pool(name=pfx + "dps", bufs=1, space="PSUM") as dps, \
             tc.tile_pool(name=pfx + "dops", bufs=2, space="PSUM") as dops:
            psA2 = dps.tile([128, R], f32, tag="psA2")
            psR2 = dps.tile([1, R], f32, tag="psR2")
            ptiles = {}
            pair = None
            for jt in range(JT):
                form = _form(jt)
                if form == "S":
                    e_t = dt_.tile([128, R], f16, tag="t1")
                    nc.scalar.activation(out=e_t, in_=g1rep, func=AF.Prelu,
                                         bias=g2j32[:, jt:jt + 1],
                                         scale=1.0, alpha=ALPHA)
                    u2 = dt_.tile([128, R], f16, tag="u2")
                    nc.scalar.activation(out=u2, in_=e_t, func=AF.Exp,
                                         bias=nSH2_b, scale=1.0)
                    p16 = dp.tile([128, R], f16, tag="p16")
                    nc.vector.tensor_tensor(out=p16, in0=u2,
                                            in1=adjT_sb[:, jt, :],
                                            op=ALU.mult)
                    ptiles[jt] = p16
                    _mm_acc(nc, psR2, ones16, p16,
                            start=(jt == 0), stop=False)
                else:
                    t1 = dt_.tile([128, R], f16, tag="t1")
                    nc.vector.tensor_scalar_mul(t1, a1repD, b1D[:, jt:jt + 1])
                    t2 = dt_.tile([128, R], f16, tag="t2")
                    nc.vector.tensor_scalar_mul(t2, a2repD, b2D[:, jt:jt + 1])
                    u2 = dt_.tile([128, R], f16, tag="u2")
                    nc.vector.tensor_tensor(out=u2, in0=t1, in1=t2, op=ALU.max)
                    if jt % 2 == 0:
                        pair = dp.tile([128, 2, PR], f8e5, tag="p2")
                        ptiles[jt] = pair
                    nc.gpsimd.tensor_tensor(out=pair[:, jt % 2, 0:R], in0=u2,
                                            in1=adjT_sb[:, jt, :],
                                            op=ALU.mult)
                    if jt % 2 == 1:
                        _mm_acc(nc, psR2, ones8[:, :, 0:1], pair[:, :, 0:R],
                                start=False, stop=False, perf_mode=DR)
                    elif jt == JT - 1:
                        _mm_acc(nc, psR2, ones8[:, 0, 0:1], pair[:, 0, 0:R],
                                start=False, stop=True)

            # load the gathered Wh2 (e4m3, j on partitions), widen the
            # S-form slices to fp16, and aggregate
            nc.vector.memset(wh2j8[:, JT - 1, :], 0.0)
            nc.sync.dma_start(
                out=wh2j8[:, 0:JT - 1, :],
                in_=_dram_ap(bass, ccW, 0,
                             [[HID, 128], [128 * HID, JT - 1], [1, HID]]))
            nc.sync.dma_start(
                out=wh2j8[:N - (JT - 1) * 128, JT - 1, :],
                in_=_dram_ap(bass, ccW, (JT - 1) * 128 * HID,
                             [[HID, N - (JT - 1) * 128], [1, HID]]))
            wh2j16S = late.tile([128, 24, HID], f16)
            for jt in range(JT):
                if _form(jt) == "S":
                    nc.vector.tensor_copy(out=wh2j16S[:, _sidx(jt), :],
                                          in_=wh2j8[:, jt, :])
            for jt in range(JT):
                if _form(jt) == "S":
                    _mm_acc(nc, psA2, wh2j16S[:, _sidx(jt), :], ptiles[jt],
                            start=(jt == 0), stop=False)
                elif jt % 2 == 1:
                    _mm_acc(nc, psA2, wh2j8[:, jt - 1:jt + 1, :],
                            ptiles[jt - 1][:, :, 0:R], start=False,
                            stop=False, perf_mode=DR)
                elif jt == JT - 1:
                    _mm_acc(nc, psA2, wh2j8[:, jt, :],
                            ptiles[jt][:, 0, 0:R], start=False, stop=True)

            lnr2 = dfin.tile([1, R], f32, tag="lnr2")
            nc.scalar.activation(out=lnr2, in_=psR2, func=AF.Ln)
            r216 = dfin.tile([1, R], f16, tag="r216")
            nc.scalar.activation(out=r216, in_=lnr2, func=AF.Exp,
                                 bias=0.0, scale=-1.0)
            nc.sync.dma_start(out=rd[1:2, :], in_=r216)
            r2rep = dfin.tile([128, R], f16, tag="r2rep")
            nc.sync.dma_start(out=r2rep, in_=_bcast_row(bass, rd[1:2, :]))
            o_t = dfin.tile([128, R], f32, tag="o")
            nc.vector.tensor_tensor(out=o_t, in0=psA2, in1=r2rep,
                                    op=ALU.mult)

            # transpose back to row layout and write out
            for it in range(6):
                w = min(128, R - it * 128)
                po = dops.tile([128, 128], f32, tag="po")
                nc.tensor.transpose(po[:w, :],
                                    o_t[:, it * 128:it * 128 + w], ident32)
                orow = dout.tile([128, 128], f32, tag="orow")
                nc.any.tensor_copy(out=orow[:w, :], in_=po[:w, :])
                nc.sync.dma_start(out=OUT[it * 128:it * 128 + w, :],
                                  in_=orow[:w, :])
        late_cm.__exit__(None, None, None)
        cwp.__exit__(None, None, None)


def _host_prep(x, adj, W_heads, a_heads, W_out, a_out):
    """Per-core input maps. Layout/pad/cast only -- no model math."""
    xT = np.zeros((KP, NPAD), np.float16)
    xT[:F_IN, :N] = x.T.astype(np.float16)
    W16 = np.zeros((KP, NH), np.float16)
    W16[:F_IN] = W_heads.transpose(1, 0, 2).reshape(F_IN, NH).astype(np.float16)
    WT32 = np.zeros((HID, H, KP), np.float16)
    WT32[:, :, :F_IN] = W_heads.transpose(2, 0, 1)
    a12 = np.stack([a_heads[:, :HID, 0], a_heads[:, HID:, 0]], axis=2)
    a12 = np.ascontiguousarray(a12.transpose(1, 0, 2)).astype(np.float16)
    aob = np.concatenate([a_out[:HID], a_out[HID:]], axis=1).astype(np.float16)
    Wout16 = W_out.astype(np.float16)

    in_maps = []
    for c in range(NC):
        rows = slice(c * R, (c + 1) * R)
        adjT = np.zeros((NPAD, R), np.float16)
        adjT[:N, :] = adj[rows].T
        xTl = np.zeros((KP, R), np.float16)
        xTl[:F_IN] = x[rows].T.astype(np.float16)
        in_maps.append({
            "xT16": xT, "xTl16": xTl, "W16": W16, "WT32": WT32, "a12": a12,
            "aob16": aob, "Wout16": Wout16,
            "adjT": np.ascontiguousarray(adjT),
        })
    return in_maps


def run(inputs, trace=False, **kw):
    from concourse.bass_utils import run_bass_kernel_spmd
    if "nc" not in _CACHE:
        _CACHE["nc"] = _build()
    nc = _CACHE["nc"]
    in_maps = _host_prep(**inputs)
    res = run_bass_kernel_spmd(nc, in_maps, core_ids=list(range(NC)),
                               trace=trace, **kw)
    out = np.concatenate([res.results[c]["out"] for c in range(NC)], axis=0)
    return out, res


def kernel(x, adj, W_heads, a_heads, W_out, a_out):
    out, _ = run(dict(x=np.asarray(x), adj=np.asarray(adj),
                      W_heads=np.asarray(W_heads), a_heads=np.asarray(a_heads),
                      W_out=np.asarray(W_out), a_out=np.asarray(a_out)))
    return out
